# revision 10
# baseline (speedup 1.0000x reference)
"""Bilateral filter (d=5, sigmaColor=0.1, sigmaSpace=1) Trainium2 Bass kernel.

Full inputs in, full outputs out. Data-parallel over 8 NeuronCores: 2 images
per core. Per-core layout: partitions = (img, row-block-of-8); each partition
stores, in fp16, a planar [ch][12 rows][Wp+4 cols] center tile whose 2-row /
2-col halos make every 5x5 window offset a pure free-dim AP shift. A second
copy shifted by one column keeps odd-dx reads 4B-aligned so fp16
tensor_tensor ops stay in the 2x packed DVE perf mode. The image is processed
in column passes of Wp=256 to fit SBUF.

Per window offset o=(dy,dx):
    diff = center - nbr(o)          (DVE, 3ch)
    sq   = diff^2                   (ACT Square, in place)
    cd   = sq_R + sq_G              (DVE)
    cd  += sq_B                     (GPSIMD)
    w    = exp(-50*cd - (dy^2+dx^2)/2)   (ACT, fused scale+bias)
    t    = w * nbr(o)               (DVE, w broadcast over ch)
    num += t                        (DVE, in place)
    den += w                        (GPSIMD, in place)
Center tap (w=1) is added at the end; 1/den via the fast approx reciprocal.
"""

import os
import sys

import numpy as np

for _p in ("/opt/trn_rl_repo",):
    if os.path.isdir(_p) and _p not in sys.path:
        sys.path.append(_p)

import concourse.bacc as bacc
import concourse.bass as bass
import concourse.tile as tile
from concourse import mybir
from concourse.ap import AP
from concourse.bass_utils import run_bass_kernel_spmd

F16 = mybir.dt.float16
F32 = mybir.dt.float32
ALU = mybir.AluOpType
ACTF = mybir.ActivationFunctionType

N_CORES = 8
R = 2  # window radius

# offsets sorted so the largest weights are accumulated last
OFFSETS = sorted(
    [(dy, dx) for dy in range(-R, R + 1) for dx in range(-R, R + 1) if (dy, dx) != (0, 0)],
    key=lambda o: -(o[0] * o[0] + o[1] * o[1]),
)


class Cfg:
    def __init__(self, B=2, H=512, W=512, Wp=256):
        self.B, self.H, self.W, self.Wp = B, H, W, Wp
        self.C = 3
        self.RBR = 8                      # core rows per partition
        self.RBN = H // self.RBR          # row blocks per image
        self.P = B * self.RBN             # partitions
        self.RH = self.RBR + 2 * R        # stored rows (12)
        self.WS = Wp + 2 * R              # stored cols per pass
        self.NPASS = W // Wp
        assert H % self.RBR == 0 and W % Wp == 0 and self.P <= 128
        assert self.WS % 2 == 0


FULL = Cfg()


def build(cfg: Cfg, enable_asserts=False, repeat=1):
    B, H, W, Wp, C = cfg.B, cfg.H, cfg.W, cfg.Wp, cfg.C
    P, RBN, RBR, RH, WS = cfg.P, cfg.RBN, cfg.RBR, cfg.RH, cfg.WS
    WC = W * C          # f32 elems per image row in DRAM
    HWC = H * WC

    nc = bacc.Bacc(
        "TRN2",
        target_bir_lowering=False,
        debug=False,
        enable_asserts=enable_asserts,
        num_devices=N_CORES,
    )
    for bv in sorted({-0.5 * float(dy * dy + dx * dx) for dy, dx in OFFSETS}):
        t = nc.alloc_sbuf_tensor(f"const-bias-{bv}", [128, 1], F32)
        nc.gpsimd.memset(t.ap(), bv)
        nc.const_aps.aps[(F32, bv)] = t.ap()
    nc.all_engine_barrier()

    x_h = nc.dram_tensor("x", [B, H, W, C], F32, kind="ExternalInput")
    y_h = nc.dram_tensor("out", [B, H, W, C], F32, kind="ExternalOutput")
    x_flat = x_h.ap().rearrange("b h w c -> (b h w c)")
    y_flat = y_h.ap().rearrange("b h w c -> (b h w c)")

    def dram_ap(flat, offset, dims):
        return AP(flat.tensor, offset, [list(d) for d in dims])

    with tile.TileContext(nc) as tc:
        with (
            tc.tile_pool(name="state", bufs=1) as state_pool,
            tc.tile_pool(name="big", bufs=2) as big_pool,
            tc.tile_pool(name="small", bufs=2) as small_pool,
        ):
            for rep in range(repeat):
              for ps0 in range(cfg.NPASS):
                ps = rep * cfg.NPASS + ps0
                c_lo = ps0 * Wp         # first core image col of this pass
                # valid image col range to load (incl. halo, clamped)
                v_lo = max(0, c_lo - R)
                v_hi = min(W, c_lo + Wp + R)
                nv = v_hi - v_lo        # cols loaded
                s_lo = v_lo - c_lo + R  # stored col of first loaded col

                Ct = state_pool.tile([P, C, RH, WS], F16, name=f"C_{ps}", tag="C")
                C1t = state_pool.tile([P, C, RH, WS], F16, name=f"C1_{ps}", tag="C1")
                num = state_pool.tile([P, C, RBR, Wp], F16, name=f"num_{ps}", tag="num")
                den = state_pool.tile([P, RBR * Wp], F32, name=f"den_{ps}", tag="den")

                # --- load + deinterleave (2 row chunks of 6 stored rows) ---
                if s_lo > 0:
                    nc.vector.memset(Ct[:, :, :, 0:s_lo], 0.0)
                if s_lo + nv < WS:
                    nc.vector.memset(Ct[:, :, :, s_lo + nv : WS], 0.0)
                nc.vector.memset(num[:, :, :, :], 0.0)
                nc.vector.memset(den[:, :], 0.0)

                zt = state_pool.tile([P, 2 * nv * C], F16, name=f"z_{ps}", tag="zt")
                nc.vector.memset(zt[:, :], 0.0)

                for ck in range(2):  # stored rows [6*ck, 6*ck+6)
                    r0 = 6 * ck
                    St = big_pool.tile([P, 6, nv * C], F16, name=f"S_{ps}_{ck}", tag="A")
                    # image row of stored row r is 8*rb - 2 + r
                    # main DMA: all rb whose 6 rows are fully in [0, H)
                    rb_a = 1 if ck == 0 else 0
                    rb_b = RBN if ck == 0 else RBN - 1
                    for img in range(B):
                        pb = img * RBN
                        row0 = 8 * rb_a - 2 + r0
                        nc.gpsimd.dma_start(
                            out=St[pb + rb_a : pb + rb_b, :, :],
                            in_=dram_ap(
                                x_flat,
                                img * HWC + row0 * WC + v_lo * C,
                                [(8 * WC, rb_b - rb_a), (WC, 6), (1, nv * C)],
                            ),
                        )
                        if ck == 0:
                            # rb=0: stored rows 2..6 <- image rows 0..4
                            nc.sync.dma_start(
                                out=St[pb : pb + 1, 0:2, :],
                                in_=zt[pb : pb + 1, :].rearrange(
                                    "p (r w) -> p r w", r=2, w=nv * C
                                ),
                            )
                            nc.gpsimd.dma_start(
                                out=St[pb : pb + 1, 2:6, :],
                                in_=dram_ap(
                                    x_flat,
                                    img * HWC + v_lo * C,
                                    [(8 * WC, 1), (WC, 4), (1, nv * C)],
                                ),
                            )
                        else:
                            # rb=RBN-1: stored rows 6..10 <- image rows H-4..H
                            pe = pb + RBN - 1
                            nc.sync.dma_start(
                                out=St[pe : pe + 1, 4:6, :],
                                in_=zt[pe : pe + 1, :].rearrange(
                                    "p (r w) -> p r w", r=2, w=nv * C
                                ),
                            )
                            nc.gpsimd.dma_start(
                                out=St[pe : pe + 1, 0:4, :],
                                in_=dram_ap(
                                    x_flat,
                                    img * HWC + (H - 4) * WC + v_lo * C,
                                    [(8 * WC, 1), (WC, 4), (1, nv * C)],
                                ),
                            )
                    # deinterleave: C[ch, r0+r, s_lo+w] = S[r, w, ch]
                    s_v = St[:, :, :].rearrange("p r (w c) -> p c r w", w=nv, c=C)
                    nc.scalar.copy(Ct[:, :, r0 : r0 + 6, s_lo : s_lo + nv], s_v)

                # --- odd-parity copy: C1[..., c] = C[..., c-1] ---
                # One contiguous flat shift per partition (1 DMA descriptor
                # run each). Row-wrap contamination only lands in col 0,
                # which is never read (odd-dx reads start at col 2).
                nc.vector.memset(C1t[:, :, 0:1, 0:2], 0.0)
                c_fl = Ct[:, :, :, :].rearrange("p c r w -> p (c r w)")
                c1_fl = C1t[:, :, :, :].rearrange("p c r w -> p (c r w)")
                fd_c = C * RH * WS
                nc.sync.dma_start(out=c1_fl[:, 1:fd_c], in_=c_fl[:, 0 : fd_c - 1])

                ctr = Ct[:, :, R : R + RBR, R : R + Wp]
                den2 = den.rearrange("p (r w) -> p r w", r=RBR, w=Wp)

                # --- 24 window offsets ---
                for dy, dx in OFFSETS:
                    if dx % 2 == 0:
                        nbr = Ct[:, :, R + dy : R + dy + RBR, R + dx : R + dx + Wp]
                    else:
                        nbr = C1t[
                            :, :, R + dy : R + dy + RBR, R + 1 + dx : R + 1 + dx + Wp
                        ]
                    D = big_pool.tile([P, C, RBR, Wp], F16, name=f"D_{ps}_{dy}_{dx}", tag="A")
                    T = big_pool.tile([P, C, RBR, Wp], F16, name=f"T_{ps}_{dy}_{dx}", tag="B")
                    cd = small_pool.tile([P, RBR, Wp], F16, name=f"cd_{ps}_{dy}_{dx}", tag="cd")
                    w = small_pool.tile([P, RBR * Wp], F16, name=f"w_{ps}_{dy}_{dx}", tag="w")
                    w3 = w.rearrange("p (r w) -> p r w", r=RBR, w=Wp)

                    nc.vector.tensor_sub(D[:, :, :, :], ctr, nbr)
                    nc.scalar.activation(D[:, :, :, :], D[:, :, :, :], ACTF.Square)
                    nc.vector.tensor_add(cd[:, :, :], D[:, 0], D[:, 1])
                    nc.gpsimd.tensor_tensor(cd[:, :, :], cd[:, :, :], D[:, 2], ALU.add)
                    nc.scalar.activation(
                        w3, cd[:, :, :], ACTF.Exp,
                        bias=-0.5 * float(dy * dy + dx * dx), scale=-50.0,
                    )
                    wb = w3.unsqueeze(1).broadcast_to((P, C, RBR, Wp))
                    nc.vector.tensor_mul(T[:, :, :, :], wb, nbr)
                    nc.vector.tensor_add(num[:, :, :, :], num[:, :, :, :], T[:, :, :, :])
                    nc.gpsimd.tensor_tensor(den2, den2, w3, ALU.add)

                # --- center tap + normalize ---
                nc.vector.tensor_add(num[:, :, :, :], num[:, :, :, :], ctr)
                nc.vector.tensor_scalar_add(den[:, :], den[:, :], 1.0)
                rden = small_pool.tile([P, RBR * Wp], F32, name=f"rden_{ps}", tag="rden")
                nc.vector.reciprocal_approx_fast(rden[:, :], den[:, :])
                rb3 = rden.rearrange("p (r w) -> p r w", r=RBR, w=Wp)
                rbb = rb3.unsqueeze(1).broadcast_to((P, C, RBR, Wp))
                nc.vector.tensor_mul(num[:, :, :, :], rbb, num[:, :, :, :])

                # --- reinterleave + store (cast fp16->f32 in DMA) ---
                Oi = big_pool.tile([P, RBR * Wp * C], F16, name=f"Oi_{ps}", tag="B")
                o_v = Oi[:, :].rearrange("p (r w c) -> p c r w", r=RBR, w=Wp, c=C)
                nc.scalar.copy(o_v, num[:, :, :, :])
                for img in range(B):
                    pb = img * RBN
                    nc.gpsimd.dma_start(
                        out=dram_ap(
                            y_flat,
                            img * HWC + c_lo * C,
                            [(8 * WC, RBN), (WC, RBR), (1, Wp * C)],
                        ),
                        in_=Oi[pb : pb + RBN, :].rearrange(
                            "p (r w) -> p r w", r=RBR, w=Wp * C
                        ),
                    )

    nc.compile()
    return nc


def make_timed_fn(nc, in_maps, n_cores=N_CORES):
    """Jitted sharded executor over device-resident inputs, no donation
    (kernel writes every output element), for wall-clock benchmarking."""
    import jax
    from jax.sharding import Mesh, PartitionSpec
    from jax.experimental.shard_map import shard_map
    import concourse.bass2jax as b2j
    from concourse import mybir as _mb

    b2j.install_neuronx_cc_hook()
    partition_name = nc.partition_id_tensor.name if nc.partition_id_tensor else None
    in_names, out_names, out_avals = [], [], []
    for alloc in nc.m.functions[0].allocations:
        if not isinstance(alloc, _mb.MemoryLocationSet):
            continue
        name = alloc.memorylocations[0].name
        if alloc.kind == "ExternalInput":
            if name != partition_name:
                in_names.append(name)
        elif alloc.kind == "ExternalOutput":
            out_names.append(name)
            out_avals.append(
                jax.core.ShapedArray(tuple(alloc.tensor_shape), _mb.dt.np(alloc.dtype))
            )
    n_params = len(in_names)
    zero_outs = [np.zeros(a.shape, a.dtype) for a in out_avals]
    all_in_names = list(in_names) + list(out_names)
    if partition_name is not None:
        all_in_names.append(partition_name)
    if nc.dbg_addr is not None:
        in_maps = [
            {**m, nc.dbg_addr.name: np.zeros((1, 2), np.uint32)} for m in in_maps
        ]
        if nc.dbg_addr.name not in in_names:
            in_names.append(nc.dbg_addr.name)
            all_in_names.insert(len(in_names) - 1, nc.dbg_addr.name)
            n_params += 1

    def _body(*args):
        operands = list(args)
        if partition_name is not None:
            operands.append(b2j.partition_id_tensor())
        return tuple(
            b2j._bass_exec_p.bind(
                *operands,
                out_avals=tuple(out_avals),
                in_names=tuple(all_in_names),
                out_names=tuple(out_names),
                lowering_input_output_aliases=(),
                sim_require_finite=True,
                sim_require_nnan=True,
                nc=nc,
            )
        )

    devices = jax.devices()[:n_cores]
    mesh = Mesh(np.asarray(devices), ("core",))
    n_outs = len(out_names)
    sharded = jax.jit(
        shard_map(
            _body,
            mesh=mesh,
            in_specs=(PartitionSpec("core"),) * (n_params + n_outs),
            out_specs=(PartitionSpec("core"),) * n_outs,
            check_rep=False,
        ),
        keep_unused=True,
    )
    concat_in = [
        np.concatenate([np.asarray(m[name]) for m in in_maps], axis=0)
        for name in in_names
    ]
    concat_zero = [
        np.zeros((n_cores * z.shape[0], *z.shape[1:]), z.dtype) for z in zero_outs
    ]
    sharding = jax.sharding.NamedSharding(mesh, PartitionSpec("core"))
    dev_args = [jax.device_put(a, sharding) for a in concat_in + concat_zero]

    def run():
        outs = sharded(*dev_args)
        jax.block_until_ready(outs)
        return outs

    return run


def bench(x=None, iters=6, repeats=(1, 5)):
    import time as _t

    if x is None:
        rng = np.random.default_rng(0)
        x = rng.random((16, 512, 512, 3), dtype=np.float32)
    x = np.ascontiguousarray(np.asarray(x), np.float32)
    bpc = x.shape[0] // N_CORES
    in_maps = [{"x": x[i * bpc : (i + 1) * bpc]} for i in range(N_CORES)]
    times = {}
    for rep in repeats:
        nc = build(FULL, repeat=rep)
        fn = make_timed_fn(nc, in_maps)
        fn()  # compile + warmup
        fn()
        ts = []
        for _ in range(iters):
            t0 = _t.perf_counter()
            fn()
            ts.append(_t.perf_counter() - t0)
        times[rep] = min(ts)
        print(f"repeat={rep}: min wall {times[rep]*1e6:.0f} us over {iters} iters")
    r0, r1 = repeats
    hw_ns = (times[r1] - times[r0]) / (r1 - r0) * 1e9
    print(f"HW exec time: {hw_ns:.0f} ns")
    return hw_ns


_NC_CACHE = {}


def _get_nc():
    if "full" not in _NC_CACHE:
        _NC_CACHE["full"] = build(FULL)
    return _NC_CACHE["full"]


def kernel(x, trace=False, **_ignored):
    x = np.ascontiguousarray(np.asarray(x), dtype=np.float32)
    B = x.shape[0]
    bpc = B // N_CORES
    nc = _get_nc()
    in_maps = [{"x": x[i * bpc : (i + 1) * bpc]} for i in range(N_CORES)]
    res = run_bass_kernel_spmd(nc, in_maps, list(range(N_CORES)), trace=trace)
    out = np.concatenate([res.results[i]["out"] for i in range(N_CORES)], axis=0)
    if trace:
        kernel.last_results = res
    return out.astype(np.float32)
